# revision 17
# baseline (speedup 1.0000x reference)
"""Trainium2 Bass kernel for nn_Eq1dConv (conv1d(K=3)+bias -> filtered_lrelu).

Math (separable along W; H untouched because the 2x up/down in H uses a
1-tap filter, so inserted zero rows are dropped again by the ::2 decimate):

  y_b[co,h,m] = sum_{ci,k} x[ci,h,m+k-1]*w[co,ci,k] + b[co]      (m in [0,512))
  A[m]  = fk1*(y_b[m-1]+y_b[m])                 (up-FIR even phase, fk1==fk3)
  Bv[m] = fk0*(y_b[m-1]+y_b[m+1]) + fk2*y_b[m]  (odd phase, fk0==fk4)
  out[n] = d0*lr(A[n]) + d1*lr(Bv[n]) + d1*lr(A[n+1]) + d0*lr(Bv[n+1])

with lr = leaky-relu(0.2), fk = 4*flip(up_filter), [d0,d1,d1,d0] = flip(down_filter)
(both FIR filters are linear-phase/symmetric).

Engine assignment (HW-measured op costs; per 4-rowpair granule):
- tensor: 12 conv matmuls only (no diagonal comb matmuls - the old baseline
  burned 57% of PE time scaling by diagonals).
- scalar ACT: single eviction y+bias -> padded z (f32 PSUM -> f16), then the
  two leaky-relus via Prelu with the filter scales folded into the ACT
  pre-scale: lrelu(c*u) == c*lr(u)-with-signs-handled for the grouped scales
  a_s = Prelu(d0*fk1 * s_a), b_s = Prelu(d1*fk0 * u). Prelu costs the same
  as Identity (~0.98 ns/elem) and is shift-insensitive.
- DVE: neighbor sums as TT (2x even with odd-element shifts - measured),
  ratio-scale as TS (4x), comb as TT + 2x STT (STT casts to f32 out free).
- gpsimd: s_b0 TT only (TS/STT unsupported or catastrophically slow there).

out[n] = G[n] + rho*a_s[n+1] + (1/rho)*b_s[n+1],  G = a_s + b_s, rho = d1/d0.

Sharding: pure data-parallel, batch 8 -> 8 cores, weights replicated.
"""

import numpy as np
from contextlib import ExitStack

import concourse.bass as bass
import concourse.bacc as bacc
import concourse.mybir as mybir
import concourse.tile as tile
from concourse.bass_utils import run_bass_kernel_spmd

B, CIN, COUT, H, W, K = 8, 64, 64, 64, 512, 3
N_CORES = 8
SLOPE = 0.2

F32 = mybir.dt.float32
F16 = mybir.dt.float16
ADD = mybir.AluOpType.add
MULT = mybir.AluOpType.mult
Prelu = mybir.ActivationFunctionType.Prelu
Ident = mybir.ActivationFunctionType.Identity


def build_program(n_rowpairs=H // 2, rp_per_gran=4):
    """Build the single-core SPMD program. Returns (nc, go)."""
    nc = bacc.Bacc("TRN2", target_bir_lowering=False, debug=False)

    x_d = nc.declare_dram_parameter("x", [CIN, H, W], F16, isOutput=False)
    wb_d = nc.declare_dram_parameter("wb", [K, 128, 128], F16, isOutput=False)
    bcol_d = nc.declare_dram_parameter("bcol", [128, 1], F32, isOutput=False)
    out_d = nc.declare_dram_parameter("out", [COUT, H, W], F16, isOutput=True)

    n_gran = (n_rowpairs + rp_per_gran - 1) // rp_per_gran
    NZB = 3  # z buffer count
    ZW = 516  # padded z width: z[0]=0, z[1+m]=y_b[m], z[513..515]=0

    def go(ratio, cA, cB, rho):
        with tile.TileContext(nc) as tc, ExitStack() as ctx:
            cpool = ctx.enter_context(tc.tile_pool(name="consts", bufs=1))
            xpool = ctx.enter_context(tc.tile_pool(name="xg", bufs=3))
            opool = ctx.enter_context(tc.tile_pool(name="og", bufs=3))
            ypool = ctx.enter_context(
                tc.tile_pool(name="ypsum", bufs=2, space=bass.MemorySpace.PSUM)
            )
            wkpool = ctx.enter_context(tc.tile_pool(name="work", bufs=3))

            wb_t = []
            for k in range(K):
                _wbt = cpool.tile([128, 128], F16, tag=f"wb{k}", name=f"wb{k}")
                wb_t.append(_wbt)
            # wb1 first: the first matmul of every rowpair uses tap k=1
            for k in (1, 0, 2):
                nc.sync.dma_start(wb_t[k][:], wb_d[k])
            bcol = cpool.tile([128, 1], F32, tag="bcol")
            nc.sync.dma_start(bcol[:], bcol_d[:])

            # dummy Prelu on a scratch column: hoists the ACT_TABLE_LOAD to
            # program start instead of the first eviction (saves ~2.5us ramp)
            warm = cpool.tile([128, 2], F16, tag="warm")
            nc.vector.memset(warm[:], 0.0)
            nc.scalar.activation(
                warm[:, 1:2], warm[:, 0:1], Prelu, bias=0.0, scale=1.0, alpha=SLOPE
            )
            # PE warmup: spin the tensor engine early so the P-state governor
            # ramps the clock before the first real granule
            wy = ypool.tile([128, 2, 512], F32, tag="y0", name="wy")
            for wi in range(12):
                nc.tensor.matmul(
                    wy[:, 0, 0:128], wb_t[1][:], wb_t[1][:],
                    start=(wi == 0), stop=(wi == 11),
                )
            nc.scalar.activation(warm[:, 0:1], wy[:, 0, 0:1], Ident, bias=0.0, scale=1.0)

            # persistent padded z buffers: [128, rp, ZW]; only cols 1:513 are
            # written each granule, pads stay zero.
            zbufs = []
            for i in range(NZB):
                t = cpool.tile([128, rp_per_gran, ZW], F16, tag=f"z{i}")
                nc.vector.memset(t[:, :, 0:1], 0.0)
                nc.vector.memset(t[:, :, 513:ZW], 0.0)
                zbufs.append(t)

            mm = lambda o_, l_, r_, s1, s2: nc.tensor.matmul(
                o_, l_, r_, start=s1, stop=s2
            )

            x_view = x_d.rearrange("c (p hh) w -> (c p) hh w", p=2)
            o_view = out_d.rearrange("c (p hh) w -> (c p) hh w", p=2)

            sizes = [2] + [rp_per_gran] * ((n_rowpairs - 8) // rp_per_gran) + [2, 2, 2]
            assert sum(sizes) == n_rowpairs
            starts = [sum(sizes[:i]) for i in range(len(sizes))]
            for g, (rp0, nrp) in enumerate(zip(starts, sizes)):
                nj = nrp
                xg = xpool.tile([128, rp_per_gran, W], F16, tag="xg")
                # x is pre-cast to f16 host-side; 2 chunks so matmuls
                # on early rowpairs start before the whole granule lands
                h_ = min(nrp, rp_per_gran // 2)
                nc.sync.dma_start(xg[:, 0:h_, :], x_view[:, rp0 : rp0 + h_, :])
                if nrp > h_:
                    nc.sync.dma_start(
                        xg[:, h_:nrp, :], x_view[:, rp0 + h_ : rp0 + nrp, :]
                    )
                og = opool.tile([128, rp_per_gran, W], F16, tag="og")
                z = zbufs[g % NZB]

                # conv: rowpair-outer; evictions batched per psum pair
                npair = (nrp + 1) // 2
                y_t = [
                    ypool.tile([128, 2, 512], F32, tag=f"y{p}", name=f"y{p}")
                    for p in range(npair)
                ]
                for j in range(nrp):
                    yv = y_t[j // 2][:, j % 2, :]
                    mm(yv[:, 0:512], wb_t[1][:], xg[:, j, 0:512], True, False)
                    mm(yv[:, 1:512], wb_t[0][:], xg[:, j, 0:511], False, False)
                    mm(yv[:, 0:511], wb_t[2][:], xg[:, j, 1:512], False, True)
                    if j % 2 == 1 or j == nrp - 1:
                        p0 = j // 2
                        k = j % 2 + 1
                        nc.scalar.activation(
                            z[:, 2 * p0 : 2 * p0 + k, 1:513],
                            y_t[p0][:, 0:k, :],
                            Ident,
                            bias=bcol[:, 0:1],
                            scale=1.0,
                        )

                # neighbor sums (TT 2x; odd-element shifts are fine)
                s_a = wkpool.tile([128, rp_per_gran, 520], F16, tag="s_a")
                nc.vector.tensor_tensor(
                    s_a[:, 0:nj, 0:513], z[:, 0:nj, 0:513], z[:, 0:nj, 1:514], ADD
                )
                s_b0 = wkpool.tile([128, rp_per_gran, 520], F16, tag="s_b0")
                nc.vector.tensor_tensor(
                    s_b0[:, 0:nj, 0:513], z[:, 0:nj, 0:513], z[:, 0:nj, 2:515], ADD
                )
                t_r = wkpool.tile([128, rp_per_gran, 520], F16, tag="t_r")
                nc.vector.tensor_scalar(
                    t_r[:, 0:nj, 0:513], z[:, 0:nj, 1:514], float(ratio), None, MULT
                )
                u = wkpool.tile([128, rp_per_gran, 520], F16, tag="u")
                nc.vector.tensor_tensor(
                    u[:, 0:nj, 0:513], t_r[:, 0:nj, 0:513], s_b0[:, 0:nj, 0:513], ADD
                )

                # scaled leaky-relus on the scalar engine
                a_s = wkpool.tile([128, rp_per_gran, 520], F16, tag="a_s")
                nc.scalar.activation(
                    a_s[:, 0:nj, 0:513], s_a[:, 0:nj, 0:513], Prelu,
                    bias=0.0, scale=float(cA), alpha=SLOPE,
                )
                b_s = wkpool.tile([128, rp_per_gran, 520], F16, tag="b_s")
                nc.scalar.activation(
                    b_s[:, 0:nj, 0:513], u[:, 0:nj, 0:513], Prelu,
                    bias=0.0, scale=float(cB), alpha=SLOPE,
                )

                # comb: out[n] = (a_s+b_s)[n] + rho*a_s[n+1] + (1/rho)*b_s[n+1]
                G = wkpool.tile([128, rp_per_gran, 520], F16, tag="G")
                nc.vector.tensor_tensor(
                    G[:, 0:nj, 0:512], a_s[:, 0:nj, 0:512], b_s[:, 0:nj, 0:512], ADD
                )
                ta = wkpool.tile([128, rp_per_gran, 520], F16, tag="ta")
                nc.scalar.activation(
                    ta[:, 0:nj, 0:512], s_a[:, 0:nj, 1:513], Prelu,
                    bias=0.0, scale=float(cA * rho), alpha=SLOPE,
                )
                o1 = wkpool.tile([128, rp_per_gran, 520], F16, tag="o1")
                nc.vector.tensor_tensor(
                    o1[:, 0:nj, 0:512], ta[:, 0:nj, 0:512], G[:, 0:nj, 0:512], ADD
                )
                tb = wkpool.tile([128, rp_per_gran, 520], F16, tag="tb")
                nc.vector.tensor_scalar(
                    tb[:, 0:nj, 0:512], b_s[:, 0:nj, 1:513], float(1.0 / rho), None, MULT
                )
                nc.vector.tensor_tensor(
                    og[:, 0:nj, :], tb[:, 0:nj, 0:512], o1[:, 0:nj, 0:512], ADD
                )

                nc.sync.dma_start(o_view[:, rp0 : rp0 + nrp, :], og[:, 0:nrp, :])

    return nc, go


def derive_consts(conv_w, bias, up_filter, down_filter):
    f = np.asarray(up_filter, dtype=np.float64).reshape(-1)
    d = np.asarray(down_filter, dtype=np.float64).reshape(-1)
    fk = (f * 4.0)[::-1]
    fd = d[::-1]
    assert abs(fk[1] - fk[3]) < 1e-6 * max(1.0, abs(fk[1])), "up filter not symmetric"
    assert abs(fk[0] - fk[4]) < 1e-6 * max(1.0, abs(fk[0])), "up filter not symmetric"
    assert abs(fd[0] - fd[3]) < 1e-6 * max(1.0, abs(fd[0])), "down filter not symmetric"
    assert abs(fd[1] - fd[2]) < 1e-6 * max(1.0, abs(fd[1])), "down filter not symmetric"
    fk0, fk1, fk2 = float(fk[0]), float(fk[1]), float(fk[2])
    d0, d1 = float(fd[0]), float(fd[1])
    assert fk0 != 0.0 and d0 != 0.0 and d1 != 0.0
    ratio = fk2 / fk0

    # partition index q = 2*ci + g (g = h-half); output partition 2*co + g
    cw = np.asarray(conv_w, dtype=np.float32)  # [co, ci, 1, K]
    wb = np.zeros((K, 128, 128), dtype=np.float16)
    for k in range(K):
        wk = cw[:, :, 0, k].T.astype(np.float16)  # [ci, co]
        wb[k, 0::2, 0::2] = wk
        wb[k, 1::2, 1::2] = wk

    bcol = np.repeat(np.asarray(bias, dtype=np.float32), 2).reshape(128, 1)

    return {
        "wb": wb,
        "bcol": bcol,
        "ratio": ratio,
        "cA": d0 * fk1,
        "cB": d1 * fk0,
        "rho": d1 / d0,
    }


_CACHE = {}


def _get_compiled(key, ratio, cA, cB, rho):
    if key in _CACHE:
        return _CACHE[key]
    nc, go = build_program()
    go(ratio, cA, cB, rho)
    nc.compile()
    _CACHE[key] = nc
    return nc


def run(x, conv_w, bias, up_filter, down_filter, trace=False, **trace_kw):
    x = np.asarray(x, dtype=np.float32)
    c = derive_consts(conv_w, bias, up_filter, down_filter)

    key = (float(c["ratio"]), float(c["cA"]), float(c["cB"]), float(c["rho"]))
    nc = _get_compiled(key, c["ratio"], c["cA"], c["cB"], c["rho"])

    in_maps = []
    for i in range(N_CORES):
        in_maps.append(
            {
                "x": np.ascontiguousarray(x[i]).astype(np.float16),
                "wb": c["wb"],
                "bcol": c["bcol"],
            }
        )
    res = run_bass_kernel_spmd(
        nc, in_maps, list(range(N_CORES)), trace=trace, **trace_kw
    )
    out = np.stack([res.results[i]["out"] for i in range(N_CORES)], axis=0)
    return out.astype(np.float32), res


def kernel(x, conv_w, bias, up_filter, down_filter):
    out, _ = run(x, conv_w, bias, up_filter, down_filter)
    return out


# revision 18
# speedup vs baseline: 1.0048x; 1.0048x over previous
"""Trainium2 Bass kernel for nn_Eq1dConv (conv1d(K=3)+bias -> filtered_lrelu).

Math (separable along W; H untouched because the 2x up/down in H uses a
1-tap filter, so inserted zero rows are dropped again by the ::2 decimate):

  y_b[co,h,m] = sum_{ci,k} x[ci,h,m+k-1]*w[co,ci,k] + b[co]      (m in [0,512))
  A[m]  = fk1*(y_b[m-1]+y_b[m])                 (up-FIR even phase, fk1==fk3)
  Bv[m] = fk0*(y_b[m-1]+y_b[m+1]) + fk2*y_b[m]  (odd phase, fk0==fk4)
  out[n] = d0*lr(A[n]) + d1*lr(Bv[n]) + d1*lr(A[n+1]) + d0*lr(Bv[n+1])

with lr = leaky-relu(0.2), fk = 4*flip(up_filter), [d0,d1,d1,d0] = flip(down_filter)
(both FIR filters are linear-phase/symmetric).

Engine assignment (HW-measured op costs; per 4-rowpair granule):
- tensor: 12 conv matmuls only (no diagonal comb matmuls - the old baseline
  burned 57% of PE time scaling by diagonals).
- scalar ACT: single eviction y+bias -> padded z (f32 PSUM -> f16), then the
  two leaky-relus via Prelu with the filter scales folded into the ACT
  pre-scale: lrelu(c*u) == c*lr(u)-with-signs-handled for the grouped scales
  a_s = Prelu(d0*fk1 * s_a), b_s = Prelu(d1*fk0 * u). Prelu costs the same
  as Identity (~0.98 ns/elem) and is shift-insensitive.
- DVE: neighbor sums as TT (2x even with odd-element shifts - measured),
  ratio-scale as TS (4x), comb as TT + 2x STT (STT casts to f32 out free).
- gpsimd: s_b0 TT only (TS/STT unsupported or catastrophically slow there).

out[n] = G[n] + rho*a_s[n+1] + (1/rho)*b_s[n+1],  G = a_s + b_s, rho = d1/d0.

Sharding: pure data-parallel, batch 8 -> 8 cores, weights replicated.
"""

import numpy as np
from contextlib import ExitStack

import concourse.bass as bass
import concourse.bacc as bacc
import concourse.mybir as mybir
import concourse.tile as tile
from concourse.bass_utils import run_bass_kernel_spmd

B, CIN, COUT, H, W, K = 8, 64, 64, 64, 512, 3
N_CORES = 8
SLOPE = 0.2

F32 = mybir.dt.float32
F16 = mybir.dt.float16
ADD = mybir.AluOpType.add
MULT = mybir.AluOpType.mult
Prelu = mybir.ActivationFunctionType.Prelu
Ident = mybir.ActivationFunctionType.Identity


def build_program(n_rowpairs=H // 2, rp_per_gran=4):
    """Build the single-core SPMD program. Returns (nc, go)."""
    nc = bacc.Bacc("TRN2", target_bir_lowering=False, debug=False)

    x_d = nc.declare_dram_parameter("x", [CIN, H, W], F16, isOutput=False)
    wb_d = nc.declare_dram_parameter("wb", [K, 128, 128], F16, isOutput=False)
    bcol_d = nc.declare_dram_parameter("bcol", [128, 1], F32, isOutput=False)
    out_d = nc.declare_dram_parameter("out", [COUT, H, W], F16, isOutput=True)

    n_gran = (n_rowpairs + rp_per_gran - 1) // rp_per_gran
    NZB = 3  # z buffer count
    ZW = 516  # padded z width: z[0]=0, z[1+m]=y_b[m], z[513..515]=0

    def go(ratio, cA, cB, rho):
        with tile.TileContext(nc) as tc, ExitStack() as ctx:
            cpool = ctx.enter_context(tc.tile_pool(name="consts", bufs=1))
            xpool = ctx.enter_context(tc.tile_pool(name="xg", bufs=3))
            opool = ctx.enter_context(tc.tile_pool(name="og", bufs=3))
            ypool = ctx.enter_context(
                tc.tile_pool(name="ypsum", bufs=2, space=bass.MemorySpace.PSUM)
            )
            wkpool = ctx.enter_context(tc.tile_pool(name="work", bufs=3))

            wb_t = []
            for k in range(K):
                _wbt = cpool.tile([128, 128], F16, tag=f"wb{k}", name=f"wb{k}")
                wb_t.append(_wbt)
            # wb1 first: the first matmul of every rowpair uses tap k=1
            for k in (1, 0, 2):
                nc.sync.dma_start(wb_t[k][:], wb_d[k])
            bcol = cpool.tile([128, 1], F32, tag="bcol")
            nc.sync.dma_start(bcol[:], bcol_d[:])

            # dummy Prelu on a scratch column: hoists the ACT_TABLE_LOAD to
            # program start instead of the first eviction (saves ~2.5us ramp)
            warm = cpool.tile([128, 2], F16, tag="warm")
            nc.vector.memset(warm[:], 0.0)
            nc.scalar.activation(
                warm[:, 1:2], warm[:, 0:1], Prelu, bias=0.0, scale=1.0, alpha=SLOPE
            )
            # PE warmup: spin the tensor engine early so the P-state governor
            # ramps the clock before the first real granule
            wy = ypool.tile([128, 2, 512], F32, tag="y0", name="wy")
            for wi in range(12):
                nc.tensor.matmul(
                    wy[:, 0, 0:128], wb_t[1][:], wb_t[1][:],
                    start=(wi == 0), stop=(wi == 11),
                )
            nc.scalar.activation(warm[:, 0:1], wy[:, 0, 0:1], Ident, bias=0.0, scale=1.0)

            # persistent padded z buffers: [128, rp, ZW]; only cols 1:513 are
            # written each granule, pads stay zero.
            zbufs = []
            for i in range(NZB):
                t = cpool.tile([128, rp_per_gran, ZW], F16, tag=f"z{i}")
                nc.vector.memset(t[:, :, 0:1], 0.0)
                nc.vector.memset(t[:, :, 513:ZW], 0.0)
                zbufs.append(t)

            mm = lambda o_, l_, r_, s1, s2: nc.tensor.matmul(
                o_, l_, r_, start=s1, stop=s2
            )

            x_view = x_d.rearrange("c (p hh) w -> (c p) hh w", p=2)
            o_view = out_d.rearrange("c (p hh) w -> (c p) hh w", p=2)

            sizes = [2] + [rp_per_gran] * ((n_rowpairs - 4) // rp_per_gran) + [2]
            assert sum(sizes) == n_rowpairs
            starts = [sum(sizes[:i]) for i in range(len(sizes))]
            for g, (rp0, nrp) in enumerate(zip(starts, sizes)):
                nj = nrp
                xg = xpool.tile([128, rp_per_gran, W], F16, tag="xg")
                # x is pre-cast to f16 host-side; 2 chunks so matmuls
                # on early rowpairs start before the whole granule lands
                h_ = min(nrp, rp_per_gran // 2)
                nc.sync.dma_start(xg[:, 0:h_, :], x_view[:, rp0 : rp0 + h_, :])
                if nrp > h_:
                    nc.sync.dma_start(
                        xg[:, h_:nrp, :], x_view[:, rp0 + h_ : rp0 + nrp, :]
                    )
                og = opool.tile([128, rp_per_gran, W], F16, tag="og")
                z = zbufs[g % NZB]

                # conv: rowpair-outer; evictions batched per psum pair
                npair = (nrp + 1) // 2
                y_t = [
                    ypool.tile([128, 2, 512], F32, tag=f"y{p}", name=f"y{p}")
                    for p in range(npair)
                ]
                for j in range(nrp):
                    yv = y_t[j // 2][:, j % 2, :]
                    mm(yv[:, 0:512], wb_t[1][:], xg[:, j, 0:512], True, False)
                    mm(yv[:, 1:512], wb_t[0][:], xg[:, j, 0:511], False, False)
                    mm(yv[:, 0:511], wb_t[2][:], xg[:, j, 1:512], False, True)
                    if j % 2 == 1 or j == nrp - 1:
                        p0 = j // 2
                        k = j % 2 + 1
                        nc.scalar.activation(
                            z[:, 2 * p0 : 2 * p0 + k, 1:513],
                            y_t[p0][:, 0:k, :],
                            Ident,
                            bias=bcol[:, 0:1],
                            scale=1.0,
                        )

                # neighbor sums (TT 2x; odd-element shifts are fine)
                s_a = wkpool.tile([128, rp_per_gran, 520], F16, tag="s_a")
                nc.vector.tensor_tensor(
                    s_a[:, 0:nj, 0:513], z[:, 0:nj, 0:513], z[:, 0:nj, 1:514], ADD
                )
                s_b0 = wkpool.tile([128, rp_per_gran, 520], F16, tag="s_b0")
                nc.vector.tensor_tensor(
                    s_b0[:, 0:nj, 0:513], z[:, 0:nj, 0:513], z[:, 0:nj, 2:515], ADD
                )
                t_r = wkpool.tile([128, rp_per_gran, 520], F16, tag="t_r")
                nc.vector.tensor_scalar(
                    t_r[:, 0:nj, 0:513], z[:, 0:nj, 1:514], float(ratio), None, MULT
                )
                u = wkpool.tile([128, rp_per_gran, 520], F16, tag="u")
                nc.vector.tensor_tensor(
                    u[:, 0:nj, 0:513], t_r[:, 0:nj, 0:513], s_b0[:, 0:nj, 0:513], ADD
                )

                # scaled leaky-relus on the scalar engine
                a_s = wkpool.tile([128, rp_per_gran, 520], F16, tag="a_s")
                nc.scalar.activation(
                    a_s[:, 0:nj, 0:513], s_a[:, 0:nj, 0:513], Prelu,
                    bias=0.0, scale=float(cA), alpha=SLOPE,
                )
                b_s = wkpool.tile([128, rp_per_gran, 520], F16, tag="b_s")
                nc.scalar.activation(
                    b_s[:, 0:nj, 0:513], u[:, 0:nj, 0:513], Prelu,
                    bias=0.0, scale=float(cB), alpha=SLOPE,
                )

                # comb: out[n] = (a_s+b_s)[n] + rho*a_s[n+1] + (1/rho)*b_s[n+1]
                G = wkpool.tile([128, rp_per_gran, 520], F16, tag="G")
                nc.vector.tensor_tensor(
                    G[:, 0:nj, 0:512], a_s[:, 0:nj, 0:512], b_s[:, 0:nj, 0:512], ADD
                )
                ta = wkpool.tile([128, rp_per_gran, 520], F16, tag="ta")
                nc.scalar.activation(
                    ta[:, 0:nj, 0:512], s_a[:, 0:nj, 1:513], Prelu,
                    bias=0.0, scale=float(cA * rho), alpha=SLOPE,
                )
                o1 = wkpool.tile([128, rp_per_gran, 520], F16, tag="o1")
                nc.vector.tensor_tensor(
                    o1[:, 0:nj, 0:512], ta[:, 0:nj, 0:512], G[:, 0:nj, 0:512], ADD
                )
                tb = wkpool.tile([128, rp_per_gran, 520], F16, tag="tb")
                nc.vector.tensor_scalar(
                    tb[:, 0:nj, 0:512], b_s[:, 0:nj, 1:513], float(1.0 / rho), None, MULT
                )
                nc.vector.tensor_tensor(
                    og[:, 0:nj, :], tb[:, 0:nj, 0:512], o1[:, 0:nj, 0:512], ADD
                )

                nc.sync.dma_start(o_view[:, rp0 : rp0 + nrp, :], og[:, 0:nrp, :])

    return nc, go


def derive_consts(conv_w, bias, up_filter, down_filter):
    f = np.asarray(up_filter, dtype=np.float64).reshape(-1)
    d = np.asarray(down_filter, dtype=np.float64).reshape(-1)
    fk = (f * 4.0)[::-1]
    fd = d[::-1]
    assert abs(fk[1] - fk[3]) < 1e-6 * max(1.0, abs(fk[1])), "up filter not symmetric"
    assert abs(fk[0] - fk[4]) < 1e-6 * max(1.0, abs(fk[0])), "up filter not symmetric"
    assert abs(fd[0] - fd[3]) < 1e-6 * max(1.0, abs(fd[0])), "down filter not symmetric"
    assert abs(fd[1] - fd[2]) < 1e-6 * max(1.0, abs(fd[1])), "down filter not symmetric"
    fk0, fk1, fk2 = float(fk[0]), float(fk[1]), float(fk[2])
    d0, d1 = float(fd[0]), float(fd[1])
    assert fk0 != 0.0 and d0 != 0.0 and d1 != 0.0
    ratio = fk2 / fk0

    # partition index q = 2*ci + g (g = h-half); output partition 2*co + g
    cw = np.asarray(conv_w, dtype=np.float32)  # [co, ci, 1, K]
    wb = np.zeros((K, 128, 128), dtype=np.float16)
    for k in range(K):
        wk = cw[:, :, 0, k].T.astype(np.float16)  # [ci, co]
        wb[k, 0::2, 0::2] = wk
        wb[k, 1::2, 1::2] = wk

    bcol = np.repeat(np.asarray(bias, dtype=np.float32), 2).reshape(128, 1)

    return {
        "wb": wb,
        "bcol": bcol,
        "ratio": ratio,
        "cA": d0 * fk1,
        "cB": d1 * fk0,
        "rho": d1 / d0,
    }


_CACHE = {}


def _get_compiled(key, ratio, cA, cB, rho):
    if key in _CACHE:
        return _CACHE[key]
    nc, go = build_program()
    go(ratio, cA, cB, rho)
    nc.compile()
    _CACHE[key] = nc
    return nc


def run(x, conv_w, bias, up_filter, down_filter, trace=False, **trace_kw):
    x = np.asarray(x, dtype=np.float32)
    c = derive_consts(conv_w, bias, up_filter, down_filter)

    key = (float(c["ratio"]), float(c["cA"]), float(c["cB"]), float(c["rho"]))
    nc = _get_compiled(key, c["ratio"], c["cA"], c["cB"], c["rho"])

    in_maps = []
    for i in range(N_CORES):
        in_maps.append(
            {
                "x": np.ascontiguousarray(x[i]).astype(np.float16),
                "wb": c["wb"],
                "bcol": c["bcol"],
            }
        )
    res = run_bass_kernel_spmd(
        nc, in_maps, list(range(N_CORES)), trace=trace, **trace_kw
    )
    out = np.stack([res.results[i]["out"] for i in range(N_CORES)], axis=0)
    return out.astype(np.float32), res


def kernel(x, conv_w, bias, up_filter, down_filter):
    out, _ = run(x, conv_w, bias, up_filter, down_filter)
    return out


# revision 19
# speedup vs baseline: 1.0122x; 1.0074x over previous
"""Trainium2 Bass kernel for nn_Eq1dConv (conv1d(K=3)+bias -> filtered_lrelu).

Math (separable along W; H untouched because the 2x up/down in H uses a
1-tap filter, so inserted zero rows are dropped again by the ::2 decimate):

  y_b[co,h,m] = sum_{ci,k} x[ci,h,m+k-1]*w[co,ci,k] + b[co]      (m in [0,512))
  A[m]  = fk1*(y_b[m-1]+y_b[m])                 (up-FIR even phase, fk1==fk3)
  Bv[m] = fk0*(y_b[m-1]+y_b[m+1]) + fk2*y_b[m]  (odd phase, fk0==fk4)
  out[n] = d0*lr(A[n]) + d1*lr(Bv[n]) + d1*lr(A[n+1]) + d0*lr(Bv[n+1])

with lr = leaky-relu(0.2), fk = 4*flip(up_filter), [d0,d1,d1,d0] = flip(down_filter)
(both FIR filters are linear-phase/symmetric).

Engine assignment (HW-measured op costs; per 4-rowpair granule):
- tensor: 12 conv matmuls only (no diagonal comb matmuls - the old baseline
  burned 57% of PE time scaling by diagonals).
- scalar ACT: single eviction y+bias -> padded z (f32 PSUM -> f16), then the
  two leaky-relus via Prelu with the filter scales folded into the ACT
  pre-scale: lrelu(c*u) == c*lr(u)-with-signs-handled for the grouped scales
  a_s = Prelu(d0*fk1 * s_a), b_s = Prelu(d1*fk0 * u). Prelu costs the same
  as Identity (~0.98 ns/elem) and is shift-insensitive.
- DVE: neighbor sums as TT (2x even with odd-element shifts - measured),
  ratio-scale as TS (4x), comb as TT + 2x STT (STT casts to f32 out free).
- gpsimd: s_b0 TT only (TS/STT unsupported or catastrophically slow there).

out[n] = G[n] + rho*a_s[n+1] + (1/rho)*b_s[n+1],  G = a_s + b_s, rho = d1/d0.

Sharding: pure data-parallel, batch 8 -> 8 cores, weights replicated.
"""

import numpy as np
from contextlib import ExitStack

import concourse.bass as bass
import concourse.bacc as bacc
import concourse.mybir as mybir
import concourse.tile as tile
from concourse.bass_utils import run_bass_kernel_spmd

B, CIN, COUT, H, W, K = 8, 64, 64, 64, 512, 3
N_CORES = 8
SLOPE = 0.2

F32 = mybir.dt.float32
F16 = mybir.dt.float16
ADD = mybir.AluOpType.add
MULT = mybir.AluOpType.mult
Prelu = mybir.ActivationFunctionType.Prelu
Ident = mybir.ActivationFunctionType.Identity


def build_program(n_rowpairs=H // 2, rp_per_gran=4):
    """Build the single-core SPMD program. Returns (nc, go)."""
    nc = bacc.Bacc("TRN2", target_bir_lowering=False, debug=False)

    x_d = nc.declare_dram_parameter("x", [CIN, H, W], F16, isOutput=False)
    wb_d = nc.declare_dram_parameter("wb", [K, 128, 128], F16, isOutput=False)
    bcol_d = nc.declare_dram_parameter("bcol", [128, 1], F32, isOutput=False)
    out_d = nc.declare_dram_parameter("out", [COUT, H, W], F16, isOutput=True)

    n_gran = (n_rowpairs + rp_per_gran - 1) // rp_per_gran
    NZB = 3  # z buffer count
    ZW = 516  # padded z width: z[0]=0, z[1+m]=y_b[m], z[513..515]=0

    def go(ratio, cA, cB, rho):
        with tile.TileContext(nc) as tc, ExitStack() as ctx:
            cpool = ctx.enter_context(tc.tile_pool(name="consts", bufs=1))
            xpool = ctx.enter_context(tc.tile_pool(name="xg", bufs=3))
            opool = ctx.enter_context(tc.tile_pool(name="og", bufs=3))
            ypool = ctx.enter_context(
                tc.tile_pool(name="ypsum", bufs=2, space=bass.MemorySpace.PSUM)
            )
            wkpool = ctx.enter_context(tc.tile_pool(name="work", bufs=3))

            wb_t = []
            for k in range(K):
                _wbt = cpool.tile([128, 128], F16, tag=f"wb{k}", name=f"wb{k}")
                wb_t.append(_wbt)
            # wb1 first: the first matmul of every rowpair uses tap k=1
            for k in (1, 0, 2):
                nc.sync.dma_start(wb_t[k][:], wb_d[k])
            bcol = cpool.tile([128, 1], F32, tag="bcol")
            nc.sync.dma_start(bcol[:], bcol_d[:])

            # dummy Prelu on a scratch column: hoists the ACT_TABLE_LOAD to
            # program start instead of the first eviction (saves ~2.5us ramp)
            warm = cpool.tile([128, 2], F16, tag="warm")
            nc.vector.memset(warm[:], 0.0)
            nc.scalar.activation(
                warm[:, 1:2], warm[:, 0:1], Prelu, bias=0.0, scale=1.0, alpha=SLOPE
            )
            # PE warmup: spin the tensor engine early so the P-state governor
            # ramps the clock before the first real granule
            wy = ypool.tile([128, 2, 512], F32, tag="y0", name="wy")
            for wi in range(4):
                nc.tensor.matmul(
                    wy[:, 0, 0:128], wb_t[1][:], wb_t[1][:],
                    start=(wi == 0), stop=(wi == 3),
                )
            nc.scalar.activation(warm[:, 0:1], wy[:, 0, 0:1], Ident, bias=0.0, scale=1.0)

            # persistent padded z buffers: [128, rp, ZW]; only cols 1:513 are
            # written each granule, pads stay zero.
            zbufs = []
            for i in range(NZB):
                t = cpool.tile([128, rp_per_gran, ZW], F16, tag=f"z{i}")
                nc.vector.memset(t[:, :, 0:1], 0.0)
                nc.vector.memset(t[:, :, 513:ZW], 0.0)
                zbufs.append(t)

            mm = lambda o_, l_, r_, s1, s2: nc.tensor.matmul(
                o_, l_, r_, start=s1, stop=s2
            )

            x_view = x_d.rearrange("c (p hh) w -> (c p) hh w", p=2)
            o_view = out_d.rearrange("c (p hh) w -> (c p) hh w", p=2)

            sizes = [2] + [rp_per_gran] * ((n_rowpairs - 4) // rp_per_gran) + [2]
            assert sum(sizes) == n_rowpairs
            starts = [sum(sizes[:i]) for i in range(len(sizes))]
            for g, (rp0, nrp) in enumerate(zip(starts, sizes)):
                nj = nrp
                xg = xpool.tile([128, rp_per_gran, W], F16, tag="xg")
                # x is pre-cast to f16 host-side; 2 chunks so matmuls
                # on early rowpairs start before the whole granule lands
                h_ = min(nrp, rp_per_gran // 2)
                nc.sync.dma_start(xg[:, 0:h_, :], x_view[:, rp0 : rp0 + h_, :])
                if nrp > h_:
                    nc.sync.dma_start(
                        xg[:, h_:nrp, :], x_view[:, rp0 + h_ : rp0 + nrp, :]
                    )
                og = opool.tile([128, rp_per_gran, W], F16, tag="og")
                z = zbufs[g % NZB]

                # conv: rowpair-outer; evictions batched per psum pair
                npair = (nrp + 1) // 2
                y_t = [
                    ypool.tile([128, 2, 512], F32, tag=f"y{p}", name=f"y{p}")
                    for p in range(npair)
                ]
                for j in range(nrp):
                    yv = y_t[j // 2][:, j % 2, :]
                    mm(yv[:, 0:512], wb_t[1][:], xg[:, j, 0:512], True, False)
                    mm(yv[:, 1:512], wb_t[0][:], xg[:, j, 0:511], False, False)
                    mm(yv[:, 0:511], wb_t[2][:], xg[:, j, 1:512], False, True)
                    if j % 2 == 1 or j == nrp - 1:
                        p0 = j // 2
                        k = j % 2 + 1
                        nc.scalar.activation(
                            z[:, 2 * p0 : 2 * p0 + k, 1:513],
                            y_t[p0][:, 0:k, :],
                            Ident,
                            bias=bcol[:, 0:1],
                            scale=1.0,
                        )

                # neighbor sums (TT 2x; odd-element shifts are fine)
                s_a = wkpool.tile([128, rp_per_gran, 520], F16, tag="s_a")
                nc.vector.tensor_tensor(
                    s_a[:, 0:nj, 0:513], z[:, 0:nj, 0:513], z[:, 0:nj, 1:514], ADD
                )
                s_b0 = wkpool.tile([128, rp_per_gran, 520], F16, tag="s_b0")
                nc.vector.tensor_tensor(
                    s_b0[:, 0:nj, 0:513], z[:, 0:nj, 0:513], z[:, 0:nj, 2:515], ADD
                )
                t_r = wkpool.tile([128, rp_per_gran, 520], F16, tag="t_r")
                nc.vector.tensor_scalar(
                    t_r[:, 0:nj, 0:513], z[:, 0:nj, 1:514], float(ratio), None, MULT
                )
                u = wkpool.tile([128, rp_per_gran, 520], F16, tag="u")
                nc.vector.tensor_tensor(
                    u[:, 0:nj, 0:513], t_r[:, 0:nj, 0:513], s_b0[:, 0:nj, 0:513], ADD
                )

                # scaled leaky-relus on the scalar engine
                a_s = wkpool.tile([128, rp_per_gran, 520], F16, tag="a_s")
                nc.scalar.activation(
                    a_s[:, 0:nj, 0:513], s_a[:, 0:nj, 0:513], Prelu,
                    bias=0.0, scale=float(cA), alpha=SLOPE,
                )
                b_s = wkpool.tile([128, rp_per_gran, 520], F16, tag="b_s")
                nc.scalar.activation(
                    b_s[:, 0:nj, 0:513], u[:, 0:nj, 0:513], Prelu,
                    bias=0.0, scale=float(cB), alpha=SLOPE,
                )

                # comb: out[n] = (a_s+b_s)[n] + rho*a_s[n+1] + (1/rho)*b_s[n+1]
                G = wkpool.tile([128, rp_per_gran, 520], F16, tag="G")
                nc.vector.tensor_tensor(
                    G[:, 0:nj, 0:512], a_s[:, 0:nj, 0:512], b_s[:, 0:nj, 0:512], ADD
                )
                ta = wkpool.tile([128, rp_per_gran, 520], F16, tag="ta")
                nc.scalar.activation(
                    ta[:, 0:nj, 0:512], s_a[:, 0:nj, 1:513], Prelu,
                    bias=0.0, scale=float(cA * rho), alpha=SLOPE,
                )
                o1 = wkpool.tile([128, rp_per_gran, 520], F16, tag="o1")
                nc.vector.tensor_tensor(
                    o1[:, 0:nj, 0:512], ta[:, 0:nj, 0:512], G[:, 0:nj, 0:512], ADD
                )
                tb = wkpool.tile([128, rp_per_gran, 520], F16, tag="tb")
                nc.vector.tensor_scalar(
                    tb[:, 0:nj, 0:512], b_s[:, 0:nj, 1:513], float(1.0 / rho), None, MULT
                )
                nc.vector.tensor_tensor(
                    og[:, 0:nj, :], tb[:, 0:nj, 0:512], o1[:, 0:nj, 0:512], ADD
                )

                nc.sync.dma_start(o_view[:, rp0 : rp0 + nrp, :], og[:, 0:nrp, :])

    return nc, go


def derive_consts(conv_w, bias, up_filter, down_filter):
    f = np.asarray(up_filter, dtype=np.float64).reshape(-1)
    d = np.asarray(down_filter, dtype=np.float64).reshape(-1)
    fk = (f * 4.0)[::-1]
    fd = d[::-1]
    assert abs(fk[1] - fk[3]) < 1e-6 * max(1.0, abs(fk[1])), "up filter not symmetric"
    assert abs(fk[0] - fk[4]) < 1e-6 * max(1.0, abs(fk[0])), "up filter not symmetric"
    assert abs(fd[0] - fd[3]) < 1e-6 * max(1.0, abs(fd[0])), "down filter not symmetric"
    assert abs(fd[1] - fd[2]) < 1e-6 * max(1.0, abs(fd[1])), "down filter not symmetric"
    fk0, fk1, fk2 = float(fk[0]), float(fk[1]), float(fk[2])
    d0, d1 = float(fd[0]), float(fd[1])
    assert fk0 != 0.0 and d0 != 0.0 and d1 != 0.0
    ratio = fk2 / fk0

    # partition index q = 2*ci + g (g = h-half); output partition 2*co + g
    cw = np.asarray(conv_w, dtype=np.float32)  # [co, ci, 1, K]
    wb = np.zeros((K, 128, 128), dtype=np.float16)
    for k in range(K):
        wk = cw[:, :, 0, k].T.astype(np.float16)  # [ci, co]
        wb[k, 0::2, 0::2] = wk
        wb[k, 1::2, 1::2] = wk

    bcol = np.repeat(np.asarray(bias, dtype=np.float32), 2).reshape(128, 1)

    return {
        "wb": wb,
        "bcol": bcol,
        "ratio": ratio,
        "cA": d0 * fk1,
        "cB": d1 * fk0,
        "rho": d1 / d0,
    }


_CACHE = {}


def _get_compiled(key, ratio, cA, cB, rho):
    if key in _CACHE:
        return _CACHE[key]
    nc, go = build_program()
    go(ratio, cA, cB, rho)
    nc.compile()
    _CACHE[key] = nc
    return nc


def run(x, conv_w, bias, up_filter, down_filter, trace=False, **trace_kw):
    x = np.asarray(x, dtype=np.float32)
    c = derive_consts(conv_w, bias, up_filter, down_filter)

    key = (float(c["ratio"]), float(c["cA"]), float(c["cB"]), float(c["rho"]))
    nc = _get_compiled(key, c["ratio"], c["cA"], c["cB"], c["rho"])

    in_maps = []
    for i in range(N_CORES):
        in_maps.append(
            {
                "x": np.ascontiguousarray(x[i]).astype(np.float16),
                "wb": c["wb"],
                "bcol": c["bcol"],
            }
        )
    res = run_bass_kernel_spmd(
        nc, in_maps, list(range(N_CORES)), trace=trace, **trace_kw
    )
    out = np.stack([res.results[i]["out"] for i in range(N_CORES)], axis=0)
    return out.astype(np.float32), res


def kernel(x, conv_w, bias, up_filter, down_filter):
    out, _ = run(x, conv_w, bias, up_filter, down_filter)
    return out


# revision 21
# speedup vs baseline: 1.0173x; 1.0050x over previous
"""Trainium2 Bass kernel for nn_Eq1dConv (conv1d(K=3)+bias -> filtered_lrelu).

Math (separable along W; H untouched because the 2x up/down in H uses a
1-tap filter, so inserted zero rows are dropped again by the ::2 decimate):

  y_b[co,h,m] = sum_{ci,k} x[ci,h,m+k-1]*w[co,ci,k] + b[co]      (m in [0,512))
  A[m]  = fk1*(y_b[m-1]+y_b[m])                 (up-FIR even phase, fk1==fk3)
  Bv[m] = fk0*(y_b[m-1]+y_b[m+1]) + fk2*y_b[m]  (odd phase, fk0==fk4)
  out[n] = d0*lr(A[n]) + d1*lr(Bv[n]) + d1*lr(A[n+1]) + d0*lr(Bv[n+1])

with lr = leaky-relu(0.2), fk = 4*flip(up_filter), [d0,d1,d1,d0] = flip(down_filter)
(both FIR filters are linear-phase/symmetric).

Engine assignment (HW-measured op costs; per 4-rowpair granule):
- tensor: 12 conv matmuls only (no diagonal comb matmuls - the old baseline
  burned 57% of PE time scaling by diagonals).
- scalar ACT: single eviction y+bias -> padded z (f32 PSUM -> f16), then the
  two leaky-relus via Prelu with the filter scales folded into the ACT
  pre-scale: lrelu(c*u) == c*lr(u)-with-signs-handled for the grouped scales
  a_s = Prelu(d0*fk1 * s_a), b_s = Prelu(d1*fk0 * u). Prelu costs the same
  as Identity (~0.98 ns/elem) and is shift-insensitive.
- DVE: neighbor sums as TT (2x even with odd-element shifts - measured),
  ratio-scale as TS (4x), comb as TT + 2x STT (STT casts to f32 out free).
- gpsimd: s_b0 TT only (TS/STT unsupported or catastrophically slow there).

out[n] = G[n] + rho*a_s[n+1] + (1/rho)*b_s[n+1],  G = a_s + b_s, rho = d1/d0.

Sharding: pure data-parallel, batch 8 -> 8 cores, weights replicated.
"""

import numpy as np
from contextlib import ExitStack

import concourse.bass as bass
import concourse.bacc as bacc
import concourse.mybir as mybir
import concourse.tile as tile
from concourse.bass_utils import run_bass_kernel_spmd

B, CIN, COUT, H, W, K = 8, 64, 64, 64, 512, 3
N_CORES = 8
SLOPE = 0.2

F32 = mybir.dt.float32
F16 = mybir.dt.float16
ADD = mybir.AluOpType.add
MULT = mybir.AluOpType.mult
Prelu = mybir.ActivationFunctionType.Prelu
Ident = mybir.ActivationFunctionType.Identity


def build_program(n_rowpairs=H // 2, rp_per_gran=4):
    """Build the single-core SPMD program. Returns (nc, go)."""
    nc = bacc.Bacc("TRN2", target_bir_lowering=False, debug=False)

    x_d = nc.declare_dram_parameter("x", [CIN, H, W], F16, isOutput=False)
    wb_d = nc.declare_dram_parameter("wb", [K, 128, 128], F16, isOutput=False)
    bcol_d = nc.declare_dram_parameter("bcol", [128, 1], F32, isOutput=False)
    out_d = nc.declare_dram_parameter("out", [COUT, H, W], F16, isOutput=True)

    n_gran = (n_rowpairs + rp_per_gran - 1) // rp_per_gran
    NZB = 3  # z buffer count
    ZW = 516  # padded z width: z[0]=0, z[1+m]=y_b[m], z[513..515]=0

    def go(ratio, cA, cB, rho):
        with tile.TileContext(nc) as tc, ExitStack() as ctx:
            cpool = ctx.enter_context(tc.tile_pool(name="consts", bufs=1))
            xpool = ctx.enter_context(tc.tile_pool(name="xg", bufs=3))
            opool = ctx.enter_context(tc.tile_pool(name="og", bufs=3))
            ypool = ctx.enter_context(
                tc.tile_pool(name="ypsum", bufs=2, space=bass.MemorySpace.PSUM)
            )
            wkpool = ctx.enter_context(tc.tile_pool(name="work", bufs=3))

            wb_t = []
            for k in range(K):
                _wbt = cpool.tile([128, 128], F16, tag=f"wb{k}", name=f"wb{k}")
                wb_t.append(_wbt)
            # wb1 first: the first matmul of every rowpair uses tap k=1
            for k in (1, 0, 2):
                nc.sync.dma_start(wb_t[k][:], wb_d[k])
            bcol = cpool.tile([128, 1], F32, tag="bcol")
            nc.sync.dma_start(bcol[:], bcol_d[:])

            # dummy Prelu on a scratch column: hoists the ACT_TABLE_LOAD to
            # program start instead of the first eviction (saves ~2.5us ramp)
            warm = cpool.tile([128, 2], F16, tag="warm")
            nc.vector.memset(warm[:], 0.0)
            nc.scalar.activation(
                warm[:, 1:2], warm[:, 0:1], Prelu, bias=0.0, scale=1.0, alpha=SLOPE
            )
            # PE warmup: spin the tensor engine early so the P-state governor
            # ramps the clock before the first real granule
            wy = ypool.tile([128, 2, 512], F32, tag="y0", name="wy")
            for wi in range(4):
                nc.tensor.matmul(
                    wy[:, 0, 0:128], wb_t[1][:], wb_t[1][:],
                    start=(wi == 0), stop=(wi == 3),
                )
            nc.scalar.activation(warm[:, 0:1], wy[:, 0, 0:1], Ident, bias=0.0, scale=1.0)

            # persistent padded z buffers: [128, rp, ZW]; only cols 1:513 are
            # written each granule, pads stay zero.
            zbufs = []
            for i in range(NZB):
                t = cpool.tile([128, rp_per_gran, ZW], F16, tag=f"z{i}")
                nc.vector.memset(t[:, :, 0:1], 0.0)
                nc.vector.memset(t[:, :, 513:ZW], 0.0)
                zbufs.append(t)

            mm = lambda o_, l_, r_, s1, s2: nc.tensor.matmul(
                o_, l_, r_, start=s1, stop=s2
            )

            x_view = x_d.rearrange("c (p hh) w -> (c p) hh w", p=2)
            o_view = out_d.rearrange("c (p hh) w -> (c p) hh w", p=2)

            sizes = [2] + [rp_per_gran] * ((n_rowpairs - 4) // rp_per_gran) + [2]
            assert sum(sizes) == n_rowpairs
            starts = [sum(sizes[:i]) for i in range(len(sizes))]
            for g, (rp0, nrp) in enumerate(zip(starts, sizes)):
                nj = nrp
                xg = xpool.tile([128, rp_per_gran, W], F16, tag="xg")
                # x is pre-cast to f16 host-side; 2 chunks so matmuls
                # on early rowpairs start before the whole granule lands
                h_ = min(nrp, rp_per_gran // 2)
                nc.gpsimd.dma_start(xg[:, 0:h_, :], x_view[:, rp0 : rp0 + h_, :])
                if nrp > h_:
                    nc.gpsimd.dma_start(
                        xg[:, h_:nrp, :], x_view[:, rp0 + h_ : rp0 + nrp, :]
                    )
                og = opool.tile([128, rp_per_gran, W], F16, tag="og")
                z = zbufs[g % NZB]

                # conv: rowpair-outer; evictions batched per psum pair
                npair = (nrp + 1) // 2
                y_t = [
                    ypool.tile([128, 2, 512], F32, tag=f"y{p}", name=f"y{p}")
                    for p in range(npair)
                ]
                for j in range(nrp):
                    yv = y_t[j // 2][:, j % 2, :]
                    mm(yv[:, 0:512], wb_t[1][:], xg[:, j, 0:512], True, False)
                    mm(yv[:, 1:512], wb_t[0][:], xg[:, j, 0:511], False, False)
                    mm(yv[:, 0:511], wb_t[2][:], xg[:, j, 1:512], False, True)
                    if j % 2 == 1 or j == nrp - 1:
                        p0 = j // 2
                        k = j % 2 + 1
                        nc.scalar.activation(
                            z[:, 2 * p0 : 2 * p0 + k, 1:513],
                            y_t[p0][:, 0:k, :],
                            Ident,
                            bias=bcol[:, 0:1],
                            scale=1.0,
                        )

                # neighbor sums (TT 2x; odd-element shifts are fine)
                s_a = wkpool.tile([128, rp_per_gran, 520], F16, tag="s_a")
                nc.vector.tensor_tensor(
                    s_a[:, 0:nj, 0:513], z[:, 0:nj, 0:513], z[:, 0:nj, 1:514], ADD
                )
                s_b0 = wkpool.tile([128, rp_per_gran, 520], F16, tag="s_b0")
                nc.vector.tensor_tensor(
                    s_b0[:, 0:nj, 0:513], z[:, 0:nj, 0:513], z[:, 0:nj, 2:515], ADD
                )
                t_r = wkpool.tile([128, rp_per_gran, 520], F16, tag="t_r")
                nc.vector.tensor_scalar(
                    t_r[:, 0:nj, 0:513], z[:, 0:nj, 1:514], float(ratio), None, MULT
                )
                u = wkpool.tile([128, rp_per_gran, 520], F16, tag="u")
                nc.vector.tensor_tensor(
                    u[:, 0:nj, 0:513], t_r[:, 0:nj, 0:513], s_b0[:, 0:nj, 0:513], ADD
                )

                # scaled leaky-relus on the scalar engine
                a_s = wkpool.tile([128, rp_per_gran, 520], F16, tag="a_s")
                nc.scalar.activation(
                    a_s[:, 0:nj, 0:513], s_a[:, 0:nj, 0:513], Prelu,
                    bias=0.0, scale=float(cA), alpha=SLOPE,
                )
                b_s = wkpool.tile([128, rp_per_gran, 520], F16, tag="b_s")
                nc.scalar.activation(
                    b_s[:, 0:nj, 0:513], u[:, 0:nj, 0:513], Prelu,
                    bias=0.0, scale=float(cB), alpha=SLOPE,
                )

                # comb: out[n] = (a_s+b_s)[n] + rho*a_s[n+1] + (1/rho)*b_s[n+1]
                G = wkpool.tile([128, rp_per_gran, 520], F16, tag="G")
                nc.vector.tensor_tensor(
                    G[:, 0:nj, 0:512], a_s[:, 0:nj, 0:512], b_s[:, 0:nj, 0:512], ADD
                )
                ta = wkpool.tile([128, rp_per_gran, 520], F16, tag="ta")
                nc.scalar.activation(
                    ta[:, 0:nj, 0:512], s_a[:, 0:nj, 1:513], Prelu,
                    bias=0.0, scale=float(cA * rho), alpha=SLOPE,
                )
                o1 = wkpool.tile([128, rp_per_gran, 520], F16, tag="o1")
                nc.vector.tensor_tensor(
                    o1[:, 0:nj, 0:512], ta[:, 0:nj, 0:512], G[:, 0:nj, 0:512], ADD
                )
                tb = wkpool.tile([128, rp_per_gran, 520], F16, tag="tb")
                nc.vector.tensor_scalar(
                    tb[:, 0:nj, 0:512], b_s[:, 0:nj, 1:513], float(1.0 / rho), None, MULT
                )
                nc.vector.tensor_tensor(
                    og[:, 0:nj, :], tb[:, 0:nj, 0:512], o1[:, 0:nj, 0:512], ADD
                )

                nc.sync.dma_start(o_view[:, rp0 : rp0 + nrp, :], og[:, 0:nrp, :])

    return nc, go


def derive_consts(conv_w, bias, up_filter, down_filter):
    f = np.asarray(up_filter, dtype=np.float64).reshape(-1)
    d = np.asarray(down_filter, dtype=np.float64).reshape(-1)
    fk = (f * 4.0)[::-1]
    fd = d[::-1]
    assert abs(fk[1] - fk[3]) < 1e-6 * max(1.0, abs(fk[1])), "up filter not symmetric"
    assert abs(fk[0] - fk[4]) < 1e-6 * max(1.0, abs(fk[0])), "up filter not symmetric"
    assert abs(fd[0] - fd[3]) < 1e-6 * max(1.0, abs(fd[0])), "down filter not symmetric"
    assert abs(fd[1] - fd[2]) < 1e-6 * max(1.0, abs(fd[1])), "down filter not symmetric"
    fk0, fk1, fk2 = float(fk[0]), float(fk[1]), float(fk[2])
    d0, d1 = float(fd[0]), float(fd[1])
    assert fk0 != 0.0 and d0 != 0.0 and d1 != 0.0
    ratio = fk2 / fk0

    # partition index q = 2*ci + g (g = h-half); output partition 2*co + g
    cw = np.asarray(conv_w, dtype=np.float32)  # [co, ci, 1, K]
    wb = np.zeros((K, 128, 128), dtype=np.float16)
    for k in range(K):
        wk = cw[:, :, 0, k].T.astype(np.float16)  # [ci, co]
        wb[k, 0::2, 0::2] = wk
        wb[k, 1::2, 1::2] = wk

    bcol = np.repeat(np.asarray(bias, dtype=np.float32), 2).reshape(128, 1)

    return {
        "wb": wb,
        "bcol": bcol,
        "ratio": ratio,
        "cA": d0 * fk1,
        "cB": d1 * fk0,
        "rho": d1 / d0,
    }


_CACHE = {}


def _get_compiled(key, ratio, cA, cB, rho):
    if key in _CACHE:
        return _CACHE[key]
    nc, go = build_program()
    go(ratio, cA, cB, rho)
    nc.compile()
    _CACHE[key] = nc
    return nc


def run(x, conv_w, bias, up_filter, down_filter, trace=False, **trace_kw):
    x = np.asarray(x, dtype=np.float32)
    c = derive_consts(conv_w, bias, up_filter, down_filter)

    key = (float(c["ratio"]), float(c["cA"]), float(c["cB"]), float(c["rho"]))
    nc = _get_compiled(key, c["ratio"], c["cA"], c["cB"], c["rho"])

    in_maps = []
    for i in range(N_CORES):
        in_maps.append(
            {
                "x": np.ascontiguousarray(x[i]).astype(np.float16),
                "wb": c["wb"],
                "bcol": c["bcol"],
            }
        )
    res = run_bass_kernel_spmd(
        nc, in_maps, list(range(N_CORES)), trace=trace, **trace_kw
    )
    out = np.stack([res.results[i]["out"] for i in range(N_CORES)], axis=0)
    return out.astype(np.float32), res


def kernel(x, conv_w, bias, up_filter, down_filter):
    out, _ = run(x, conv_w, bias, up_filter, down_filter)
    return out


# revision 22
# speedup vs baseline: 1.0221x; 1.0048x over previous
"""Trainium2 Bass kernel for nn_Eq1dConv (conv1d(K=3)+bias -> filtered_lrelu).

Math (separable along W; H untouched because the 2x up/down in H uses a
1-tap filter, so inserted zero rows are dropped again by the ::2 decimate):

  y_b[co,h,m] = sum_{ci,k} x[ci,h,m+k-1]*w[co,ci,k] + b[co]      (m in [0,512))
  A[m]  = fk1*(y_b[m-1]+y_b[m])                 (up-FIR even phase, fk1==fk3)
  Bv[m] = fk0*(y_b[m-1]+y_b[m+1]) + fk2*y_b[m]  (odd phase, fk0==fk4)
  out[n] = d0*lr(A[n]) + d1*lr(Bv[n]) + d1*lr(A[n+1]) + d0*lr(Bv[n+1])

with lr = leaky-relu(0.2), fk = 4*flip(up_filter), [d0,d1,d1,d0] = flip(down_filter)
(both FIR filters are linear-phase/symmetric).

Engine assignment (HW-measured op costs; per 4-rowpair granule):
- tensor: 12 conv matmuls only (no diagonal comb matmuls - the old baseline
  burned 57% of PE time scaling by diagonals).
- scalar ACT: single eviction y+bias -> padded z (f32 PSUM -> f16), then the
  two leaky-relus via Prelu with the filter scales folded into the ACT
  pre-scale: lrelu(c*u) == c*lr(u)-with-signs-handled for the grouped scales
  a_s = Prelu(d0*fk1 * s_a), b_s = Prelu(d1*fk0 * u). Prelu costs the same
  as Identity (~0.98 ns/elem) and is shift-insensitive.
- DVE: neighbor sums as TT (2x even with odd-element shifts - measured),
  ratio-scale as TS (4x), comb as TT + 2x STT (STT casts to f32 out free).
- gpsimd: s_b0 TT only (TS/STT unsupported or catastrophically slow there).

out[n] = G[n] + rho*a_s[n+1] + (1/rho)*b_s[n+1],  G = a_s + b_s, rho = d1/d0.

Sharding: pure data-parallel, batch 8 -> 8 cores, weights replicated.
"""

import numpy as np
from contextlib import ExitStack

import concourse.bass as bass
import concourse.bacc as bacc
import concourse.mybir as mybir
import concourse.tile as tile
from concourse.bass_utils import run_bass_kernel_spmd

B, CIN, COUT, H, W, K = 8, 64, 64, 64, 512, 3
N_CORES = 8
SLOPE = 0.2

F32 = mybir.dt.float32
F16 = mybir.dt.float16
ADD = mybir.AluOpType.add
MULT = mybir.AluOpType.mult
Prelu = mybir.ActivationFunctionType.Prelu
Ident = mybir.ActivationFunctionType.Identity


def build_program(n_rowpairs=H // 2, rp_per_gran=4):
    """Build the single-core SPMD program. Returns (nc, go)."""
    nc = bacc.Bacc("TRN2", target_bir_lowering=False, debug=False)

    x_d = nc.declare_dram_parameter("x", [CIN, H, W], F16, isOutput=False)
    wb_d = nc.declare_dram_parameter("wb", [K, 128, 128], F16, isOutput=False)
    bcol_d = nc.declare_dram_parameter("bcol", [128, 1], F32, isOutput=False)
    out_d = nc.declare_dram_parameter("out", [COUT, H, W], F16, isOutput=True)

    n_gran = (n_rowpairs + rp_per_gran - 1) // rp_per_gran
    NZB = 4  # z buffer count
    ZW = 516  # padded z width: z[0]=0, z[1+m]=y_b[m], z[513..515]=0

    def go(ratio, cA, cB, rho):
        with tile.TileContext(nc) as tc, ExitStack() as ctx:
            cpool = ctx.enter_context(tc.tile_pool(name="consts", bufs=1))
            xpool = ctx.enter_context(tc.tile_pool(name="xg", bufs=3))
            opool = ctx.enter_context(tc.tile_pool(name="og", bufs=3))
            ypool = ctx.enter_context(
                tc.tile_pool(name="ypsum", bufs=2, space=bass.MemorySpace.PSUM)
            )
            wkpool = ctx.enter_context(tc.tile_pool(name="work", bufs=3))

            wb_t = []
            for k in range(K):
                _wbt = cpool.tile([128, 128], F16, tag=f"wb{k}", name=f"wb{k}")
                wb_t.append(_wbt)
            # wb1 first: the first matmul of every rowpair uses tap k=1
            for k in (1, 0, 2):
                nc.sync.dma_start(wb_t[k][:], wb_d[k])
            bcol = cpool.tile([128, 1], F32, tag="bcol")
            nc.sync.dma_start(bcol[:], bcol_d[:])

            # dummy Prelu on a scratch column: hoists the ACT_TABLE_LOAD to
            # program start instead of the first eviction (saves ~2.5us ramp)
            warm = cpool.tile([128, 2], F16, tag="warm")
            nc.vector.memset(warm[:], 0.0)
            nc.scalar.activation(
                warm[:, 1:2], warm[:, 0:1], Prelu, bias=0.0, scale=1.0, alpha=SLOPE
            )
            # PE warmup: spin the tensor engine early so the P-state governor
            # ramps the clock before the first real granule
            wy = ypool.tile([128, 4, 512], F32, tag="y0", name="wy")
            for wi in range(4):
                nc.tensor.matmul(
                    wy[:, 0, 0:128], wb_t[1][:], wb_t[1][:],
                    start=(wi == 0), stop=(wi == 3),
                )
            nc.scalar.activation(warm[:, 0:1], wy[:, 0, 0:1], Ident, bias=0.0, scale=1.0)

            # persistent padded z buffers: [128, rp, ZW]; only cols 1:513 are
            # written each granule, pads stay zero.
            zbufs = []
            for i in range(NZB):
                t = cpool.tile([128, rp_per_gran, ZW], F16, tag=f"z{i}")
                nc.vector.memset(t[:, :, 0:1], 0.0)
                nc.vector.memset(t[:, :, 513:ZW], 0.0)
                zbufs.append(t)

            mm = lambda o_, l_, r_, s1, s2: nc.tensor.matmul(
                o_, l_, r_, start=s1, stop=s2
            )

            x_view = x_d.rearrange("c (p hh) w -> (c p) hh w", p=2)
            o_view = out_d.rearrange("c (p hh) w -> (c p) hh w", p=2)

            sizes = [2] + [rp_per_gran] * ((n_rowpairs - 4) // rp_per_gran) + [2]
            assert sum(sizes) == n_rowpairs
            starts = [sum(sizes[:i]) for i in range(len(sizes))]
            for g, (rp0, nrp) in enumerate(zip(starts, sizes)):
                nj = nrp
                xg = xpool.tile([128, rp_per_gran, W], F16, tag="xg")
                # x is pre-cast to f16 host-side; 2 chunks so matmuls
                # on early rowpairs start before the whole granule lands
                h_ = min(nrp, rp_per_gran // 2)
                nc.gpsimd.dma_start(xg[:, 0:h_, :], x_view[:, rp0 : rp0 + h_, :])
                if nrp > h_:
                    nc.gpsimd.dma_start(
                        xg[:, h_:nrp, :], x_view[:, rp0 + h_ : rp0 + nrp, :]
                    )
                og = opool.tile([128, rp_per_gran, W], F16, tag="og")
                z = zbufs[g % NZB]

                # conv: rowpair-outer; single eviction per granule
                yq = ypool.tile([128, 4, 512], F32, tag="y0", name="yq")
                for j in range(nrp):
                    yv = yq[:, j, :]
                    mm(yv[:, 0:512], wb_t[1][:], xg[:, j, 0:512], True, False)
                    mm(yv[:, 1:512], wb_t[0][:], xg[:, j, 0:511], False, False)
                    mm(yv[:, 0:511], wb_t[2][:], xg[:, j, 1:512], False, True)
                nc.scalar.activation(
                    z[:, 0:nrp, 1:513],
                    yq[:, 0:nrp, :],
                    Ident,
                    bias=bcol[:, 0:1],
                    scale=1.0,
                )

                # neighbor sums (TT 2x; odd-element shifts are fine)
                s_a = wkpool.tile([128, rp_per_gran, 520], F16, tag="s_a")
                nc.vector.tensor_tensor(
                    s_a[:, 0:nj, 0:513], z[:, 0:nj, 0:513], z[:, 0:nj, 1:514], ADD
                )
                s_b0 = wkpool.tile([128, rp_per_gran, 520], F16, tag="s_b0")
                nc.vector.tensor_tensor(
                    s_b0[:, 0:nj, 0:513], z[:, 0:nj, 0:513], z[:, 0:nj, 2:515], ADD
                )
                t_r = wkpool.tile([128, rp_per_gran, 520], F16, tag="t_r")
                nc.vector.tensor_scalar(
                    t_r[:, 0:nj, 0:513], z[:, 0:nj, 1:514], float(ratio), None, MULT
                )
                u = wkpool.tile([128, rp_per_gran, 520], F16, tag="u")
                nc.vector.tensor_tensor(
                    u[:, 0:nj, 0:513], t_r[:, 0:nj, 0:513], s_b0[:, 0:nj, 0:513], ADD
                )

                # scaled leaky-relus on the scalar engine
                a_s = wkpool.tile([128, rp_per_gran, 520], F16, tag="a_s")
                nc.scalar.activation(
                    a_s[:, 0:nj, 0:513], s_a[:, 0:nj, 0:513], Prelu,
                    bias=0.0, scale=float(cA), alpha=SLOPE,
                )
                b_s = wkpool.tile([128, rp_per_gran, 520], F16, tag="b_s")
                nc.scalar.activation(
                    b_s[:, 0:nj, 0:513], u[:, 0:nj, 0:513], Prelu,
                    bias=0.0, scale=float(cB), alpha=SLOPE,
                )

                # comb: out[n] = (a_s+b_s)[n] + rho*a_s[n+1] + (1/rho)*b_s[n+1]
                G = wkpool.tile([128, rp_per_gran, 520], F16, tag="G")
                nc.vector.tensor_tensor(
                    G[:, 0:nj, 0:512], a_s[:, 0:nj, 0:512], b_s[:, 0:nj, 0:512], ADD
                )
                ta = wkpool.tile([128, rp_per_gran, 520], F16, tag="ta")
                nc.scalar.activation(
                    ta[:, 0:nj, 0:512], s_a[:, 0:nj, 1:513], Prelu,
                    bias=0.0, scale=float(cA * rho), alpha=SLOPE,
                )
                o1 = wkpool.tile([128, rp_per_gran, 520], F16, tag="o1")
                nc.vector.tensor_tensor(
                    o1[:, 0:nj, 0:512], ta[:, 0:nj, 0:512], G[:, 0:nj, 0:512], ADD
                )
                tb = wkpool.tile([128, rp_per_gran, 520], F16, tag="tb")
                nc.vector.tensor_scalar(
                    tb[:, 0:nj, 0:512], b_s[:, 0:nj, 1:513], float(1.0 / rho), None, MULT
                )
                nc.vector.tensor_tensor(
                    og[:, 0:nj, :], tb[:, 0:nj, 0:512], o1[:, 0:nj, 0:512], ADD
                )

                nc.sync.dma_start(o_view[:, rp0 : rp0 + nrp, :], og[:, 0:nrp, :])

    return nc, go


def derive_consts(conv_w, bias, up_filter, down_filter):
    f = np.asarray(up_filter, dtype=np.float64).reshape(-1)
    d = np.asarray(down_filter, dtype=np.float64).reshape(-1)
    fk = (f * 4.0)[::-1]
    fd = d[::-1]
    assert abs(fk[1] - fk[3]) < 1e-6 * max(1.0, abs(fk[1])), "up filter not symmetric"
    assert abs(fk[0] - fk[4]) < 1e-6 * max(1.0, abs(fk[0])), "up filter not symmetric"
    assert abs(fd[0] - fd[3]) < 1e-6 * max(1.0, abs(fd[0])), "down filter not symmetric"
    assert abs(fd[1] - fd[2]) < 1e-6 * max(1.0, abs(fd[1])), "down filter not symmetric"
    fk0, fk1, fk2 = float(fk[0]), float(fk[1]), float(fk[2])
    d0, d1 = float(fd[0]), float(fd[1])
    assert fk0 != 0.0 and d0 != 0.0 and d1 != 0.0
    ratio = fk2 / fk0

    # partition index q = 2*ci + g (g = h-half); output partition 2*co + g
    cw = np.asarray(conv_w, dtype=np.float32)  # [co, ci, 1, K]
    wb = np.zeros((K, 128, 128), dtype=np.float16)
    for k in range(K):
        wk = cw[:, :, 0, k].T.astype(np.float16)  # [ci, co]
        wb[k, 0::2, 0::2] = wk
        wb[k, 1::2, 1::2] = wk

    bcol = np.repeat(np.asarray(bias, dtype=np.float32), 2).reshape(128, 1)

    return {
        "wb": wb,
        "bcol": bcol,
        "ratio": ratio,
        "cA": d0 * fk1,
        "cB": d1 * fk0,
        "rho": d1 / d0,
    }


_CACHE = {}


def _get_compiled(key, ratio, cA, cB, rho):
    if key in _CACHE:
        return _CACHE[key]
    nc, go = build_program()
    go(ratio, cA, cB, rho)
    nc.compile()
    _CACHE[key] = nc
    return nc


def run(x, conv_w, bias, up_filter, down_filter, trace=False, **trace_kw):
    x = np.asarray(x, dtype=np.float32)
    c = derive_consts(conv_w, bias, up_filter, down_filter)

    key = (float(c["ratio"]), float(c["cA"]), float(c["cB"]), float(c["rho"]))
    nc = _get_compiled(key, c["ratio"], c["cA"], c["cB"], c["rho"])

    in_maps = []
    for i in range(N_CORES):
        in_maps.append(
            {
                "x": np.ascontiguousarray(x[i]).astype(np.float16),
                "wb": c["wb"],
                "bcol": c["bcol"],
            }
        )
    res = run_bass_kernel_spmd(
        nc, in_maps, list(range(N_CORES)), trace=trace, **trace_kw
    )
    out = np.stack([res.results[i]["out"] for i in range(N_CORES)], axis=0)
    return out.astype(np.float32), res


def kernel(x, conv_w, bias, up_filter, down_filter):
    out, _ = run(x, conv_w, bias, up_filter, down_filter)
    return out


# revision 23
# speedup vs baseline: 1.0346x; 1.0122x over previous
"""Trainium2 Bass kernel for nn_Eq1dConv (conv1d(K=3)+bias -> filtered_lrelu).

Math (separable along W; H untouched because the 2x up/down in H uses a
1-tap filter, so inserted zero rows are dropped again by the ::2 decimate):

  y_b[co,h,m] = sum_{ci,k} x[ci,h,m+k-1]*w[co,ci,k] + b[co]      (m in [0,512))
  A[m]  = fk1*(y_b[m-1]+y_b[m])                 (up-FIR even phase, fk1==fk3)
  Bv[m] = fk0*(y_b[m-1]+y_b[m+1]) + fk2*y_b[m]  (odd phase, fk0==fk4)
  out[n] = d0*lr(A[n]) + d1*lr(Bv[n]) + d1*lr(A[n+1]) + d0*lr(Bv[n+1])

with lr = leaky-relu(0.2), fk = 4*flip(up_filter), [d0,d1,d1,d0] = flip(down_filter)
(both FIR filters are linear-phase/symmetric).

Engine assignment (HW-measured op costs; per 4-rowpair granule):
- tensor: 12 conv matmuls only (no diagonal comb matmuls - the old baseline
  burned 57% of PE time scaling by diagonals).
- scalar ACT: single eviction y+bias -> padded z (f32 PSUM -> f16), then the
  two leaky-relus via Prelu with the filter scales folded into the ACT
  pre-scale: lrelu(c*u) == c*lr(u)-with-signs-handled for the grouped scales
  a_s = Prelu(d0*fk1 * s_a), b_s = Prelu(d1*fk0 * u). Prelu costs the same
  as Identity (~0.98 ns/elem) and is shift-insensitive.
- DVE: neighbor sums as TT (2x even with odd-element shifts - measured),
  ratio-scale as TS (4x), comb as TT + 2x STT (STT casts to f32 out free).
- gpsimd: s_b0 TT only (TS/STT unsupported or catastrophically slow there).

out[n] = G[n] + rho*a_s[n+1] + (1/rho)*b_s[n+1],  G = a_s + b_s, rho = d1/d0.

Sharding: pure data-parallel, batch 8 -> 8 cores, weights replicated.
"""

import numpy as np
from contextlib import ExitStack

import concourse.bass as bass
import concourse.bacc as bacc
import concourse.mybir as mybir
import concourse.tile as tile
from concourse.bass_utils import run_bass_kernel_spmd

B, CIN, COUT, H, W, K = 8, 64, 64, 64, 512, 3
N_CORES = 8
SLOPE = 0.2

F32 = mybir.dt.float32
F16 = mybir.dt.float16
ADD = mybir.AluOpType.add
MULT = mybir.AluOpType.mult
Prelu = mybir.ActivationFunctionType.Prelu
Ident = mybir.ActivationFunctionType.Identity


def build_program(n_rowpairs=H // 2, rp_per_gran=4):
    """Build the single-core SPMD program. Returns (nc, go)."""
    nc = bacc.Bacc("TRN2", target_bir_lowering=False, debug=False)

    x_d = nc.declare_dram_parameter("x", [CIN, H, W], F16, isOutput=False)
    wb_d = nc.declare_dram_parameter("wb", [K, 128, 128], F16, isOutput=False)
    bcol_d = nc.declare_dram_parameter("bcol", [128, 1], F32, isOutput=False)
    out_d = nc.declare_dram_parameter("out", [COUT, H, W], F16, isOutput=True)

    n_gran = (n_rowpairs + rp_per_gran - 1) // rp_per_gran
    NZB = 4  # z buffer count
    ZW = 516  # padded z width: z[0]=0, z[1+m]=y_b[m], z[513..515]=0

    def go(ratio, cA, cB, rho):
        with tile.TileContext(nc) as tc, ExitStack() as ctx:
            cpool = ctx.enter_context(tc.tile_pool(name="consts", bufs=1))
            xpool = ctx.enter_context(tc.tile_pool(name="xg", bufs=3))
            opool = ctx.enter_context(tc.tile_pool(name="og", bufs=3))
            ypool = ctx.enter_context(
                tc.tile_pool(name="ypsum", bufs=2, space=bass.MemorySpace.PSUM)
            )
            wkpool = ctx.enter_context(tc.tile_pool(name="work", bufs=3))

            wb_t = []
            for k in range(K):
                _wbt = cpool.tile([128, 128], F16, tag=f"wb{k}", name=f"wb{k}")
                wb_t.append(_wbt)
            # wb1 first: the first matmul of every rowpair uses tap k=1
            for k in (1, 0, 2):
                nc.sync.dma_start(wb_t[k][:], wb_d[k])
            bcol = cpool.tile([128, 1], F32, tag="bcol")
            nc.sync.dma_start(bcol[:], bcol_d[:])

            # dummy Prelu on a scratch column: hoists the ACT_TABLE_LOAD to
            # program start instead of the first eviction (saves ~2.5us ramp)
            warm = cpool.tile([128, 2], F16, tag="warm")
            nc.vector.memset(warm[:], 0.0)
            nc.scalar.activation(
                warm[:, 1:2], warm[:, 0:1], Prelu, bias=0.0, scale=1.0, alpha=SLOPE
            )
            # PE warmup: spin the tensor engine early so the P-state governor
            # ramps the clock before the first real granule
            wy = ypool.tile([128, 4, 512], F32, tag="y0", name="wy")
            for wi in range(4):
                nc.tensor.matmul(
                    wy[:, 0, 0:128], wb_t[1][:], wb_t[1][:],
                    start=(wi == 0), stop=(wi == 3),
                )
            nc.scalar.activation(warm[:, 0:1], wy[:, 0, 0:1], Ident, bias=0.0, scale=1.0)

            # persistent padded z buffers: [128, rp, ZW]; only cols 1:513 are
            # written each granule, pads stay zero.
            zbufs = []
            for i in range(NZB):
                t = cpool.tile([128, rp_per_gran, ZW], F16, tag=f"z{i}")
                nc.vector.memset(t[:, :, 0:1], 0.0)
                nc.vector.memset(t[:, :, 513:ZW], 0.0)
                zbufs.append(t)

            mm = lambda o_, l_, r_, s1, s2: nc.tensor.matmul(
                o_, l_, r_, start=s1, stop=s2
            )

            x_view = x_d.rearrange("c (p hh) w -> (c p) hh w", p=2)
            o_view = out_d.rearrange("c (p hh) w -> (c p) hh w", p=2)

            sizes = [2, 2] + [rp_per_gran] * ((n_rowpairs - 8) // rp_per_gran) + [2, 2]
            assert sum(sizes) == n_rowpairs
            starts = [sum(sizes[:i]) for i in range(len(sizes))]
            for g, (rp0, nrp) in enumerate(zip(starts, sizes)):
                nj = nrp
                xg = xpool.tile([128, rp_per_gran, W], F16, tag="xg")
                # x is pre-cast to f16 host-side; 2 chunks so matmuls
                # on early rowpairs start before the whole granule lands
                h_ = min(nrp, rp_per_gran // 2)
                nc.gpsimd.dma_start(xg[:, 0:h_, :], x_view[:, rp0 : rp0 + h_, :])
                if nrp > h_:
                    nc.gpsimd.dma_start(
                        xg[:, h_:nrp, :], x_view[:, rp0 + h_ : rp0 + nrp, :]
                    )
                og = opool.tile([128, rp_per_gran, W], F16, tag="og")
                z = zbufs[g % NZB]

                # conv: rowpair-outer; single eviction per granule
                yq = ypool.tile([128, 4, 512], F32, tag="y0", name="yq")
                for j in range(nrp):
                    yv = yq[:, j, :]
                    mm(yv[:, 0:512], wb_t[1][:], xg[:, j, 0:512], True, False)
                    mm(yv[:, 1:512], wb_t[0][:], xg[:, j, 0:511], False, False)
                    mm(yv[:, 0:511], wb_t[2][:], xg[:, j, 1:512], False, True)
                nc.scalar.activation(
                    z[:, 0:nrp, 1:513],
                    yq[:, 0:nrp, :],
                    Ident,
                    bias=bcol[:, 0:1],
                    scale=1.0,
                )

                # neighbor sums (TT 2x; odd-element shifts are fine)
                s_a = wkpool.tile([128, rp_per_gran, 520], F16, tag="s_a")
                nc.vector.tensor_tensor(
                    s_a[:, 0:nj, 0:513], z[:, 0:nj, 0:513], z[:, 0:nj, 1:514], ADD
                )
                s_b0 = wkpool.tile([128, rp_per_gran, 520], F16, tag="s_b0")
                nc.vector.tensor_tensor(
                    s_b0[:, 0:nj, 0:513], z[:, 0:nj, 0:513], z[:, 0:nj, 2:515], ADD
                )
                t_r = wkpool.tile([128, rp_per_gran, 520], F16, tag="t_r")
                nc.vector.tensor_scalar(
                    t_r[:, 0:nj, 0:513], z[:, 0:nj, 1:514], float(ratio), None, MULT
                )
                u = wkpool.tile([128, rp_per_gran, 520], F16, tag="u")
                nc.vector.tensor_tensor(
                    u[:, 0:nj, 0:513], t_r[:, 0:nj, 0:513], s_b0[:, 0:nj, 0:513], ADD
                )

                # scaled leaky-relus on the scalar engine
                a_s = wkpool.tile([128, rp_per_gran, 520], F16, tag="a_s")
                nc.scalar.activation(
                    a_s[:, 0:nj, 0:513], s_a[:, 0:nj, 0:513], Prelu,
                    bias=0.0, scale=float(cA), alpha=SLOPE,
                )
                b_s = wkpool.tile([128, rp_per_gran, 520], F16, tag="b_s")
                nc.scalar.activation(
                    b_s[:, 0:nj, 0:513], u[:, 0:nj, 0:513], Prelu,
                    bias=0.0, scale=float(cB), alpha=SLOPE,
                )

                # comb: out[n] = (a_s+b_s)[n] + rho*a_s[n+1] + (1/rho)*b_s[n+1]
                G = wkpool.tile([128, rp_per_gran, 520], F16, tag="G")
                nc.vector.tensor_tensor(
                    G[:, 0:nj, 0:512], a_s[:, 0:nj, 0:512], b_s[:, 0:nj, 0:512], ADD
                )
                ta = wkpool.tile([128, rp_per_gran, 520], F16, tag="ta")
                nc.scalar.activation(
                    ta[:, 0:nj, 0:512], s_a[:, 0:nj, 1:513], Prelu,
                    bias=0.0, scale=float(cA * rho), alpha=SLOPE,
                )
                o1 = wkpool.tile([128, rp_per_gran, 520], F16, tag="o1")
                nc.vector.tensor_tensor(
                    o1[:, 0:nj, 0:512], ta[:, 0:nj, 0:512], G[:, 0:nj, 0:512], ADD
                )
                tb = wkpool.tile([128, rp_per_gran, 520], F16, tag="tb")
                nc.vector.tensor_scalar(
                    tb[:, 0:nj, 0:512], b_s[:, 0:nj, 1:513], float(1.0 / rho), None, MULT
                )
                nc.vector.tensor_tensor(
                    og[:, 0:nj, :], tb[:, 0:nj, 0:512], o1[:, 0:nj, 0:512], ADD
                )

                nc.sync.dma_start(o_view[:, rp0 : rp0 + nrp, :], og[:, 0:nrp, :])

    return nc, go


def derive_consts(conv_w, bias, up_filter, down_filter):
    f = np.asarray(up_filter, dtype=np.float64).reshape(-1)
    d = np.asarray(down_filter, dtype=np.float64).reshape(-1)
    fk = (f * 4.0)[::-1]
    fd = d[::-1]
    assert abs(fk[1] - fk[3]) < 1e-6 * max(1.0, abs(fk[1])), "up filter not symmetric"
    assert abs(fk[0] - fk[4]) < 1e-6 * max(1.0, abs(fk[0])), "up filter not symmetric"
    assert abs(fd[0] - fd[3]) < 1e-6 * max(1.0, abs(fd[0])), "down filter not symmetric"
    assert abs(fd[1] - fd[2]) < 1e-6 * max(1.0, abs(fd[1])), "down filter not symmetric"
    fk0, fk1, fk2 = float(fk[0]), float(fk[1]), float(fk[2])
    d0, d1 = float(fd[0]), float(fd[1])
    assert fk0 != 0.0 and d0 != 0.0 and d1 != 0.0
    ratio = fk2 / fk0

    # partition index q = 2*ci + g (g = h-half); output partition 2*co + g
    cw = np.asarray(conv_w, dtype=np.float32)  # [co, ci, 1, K]
    wb = np.zeros((K, 128, 128), dtype=np.float16)
    for k in range(K):
        wk = cw[:, :, 0, k].T.astype(np.float16)  # [ci, co]
        wb[k, 0::2, 0::2] = wk
        wb[k, 1::2, 1::2] = wk

    bcol = np.repeat(np.asarray(bias, dtype=np.float32), 2).reshape(128, 1)

    return {
        "wb": wb,
        "bcol": bcol,
        "ratio": ratio,
        "cA": d0 * fk1,
        "cB": d1 * fk0,
        "rho": d1 / d0,
    }


_CACHE = {}


def _get_compiled(key, ratio, cA, cB, rho):
    if key in _CACHE:
        return _CACHE[key]
    nc, go = build_program()
    go(ratio, cA, cB, rho)
    nc.compile()
    _CACHE[key] = nc
    return nc


def run(x, conv_w, bias, up_filter, down_filter, trace=False, **trace_kw):
    x = np.asarray(x, dtype=np.float32)
    c = derive_consts(conv_w, bias, up_filter, down_filter)

    key = (float(c["ratio"]), float(c["cA"]), float(c["cB"]), float(c["rho"]))
    nc = _get_compiled(key, c["ratio"], c["cA"], c["cB"], c["rho"])

    in_maps = []
    for i in range(N_CORES):
        in_maps.append(
            {
                "x": np.ascontiguousarray(x[i]).astype(np.float16),
                "wb": c["wb"],
                "bcol": c["bcol"],
            }
        )
    res = run_bass_kernel_spmd(
        nc, in_maps, list(range(N_CORES)), trace=trace, **trace_kw
    )
    out = np.stack([res.results[i]["out"] for i in range(N_CORES)], axis=0)
    return out.astype(np.float32), res


def kernel(x, conv_w, bias, up_filter, down_filter):
    out, _ = run(x, conv_w, bias, up_filter, down_filter)
    return out
